# revision 25
# baseline (speedup 1.0000x reference)
"""Bahdanau additive attention between two sequences on 8 Trainium2 NeuronCores.

Reference computation (per batch b):
    s0 = q0 @ W1[:D]          # [L, O]
    s1 = q1 @ W1[D:]          # [L, O]
    h[i,j,:]   = tanh(s1[i] + s0[j] + b1)          # [L, L, O]
    attn[i,j]  = h[i,j,:] . W2 (+ b2, drops out of softmax)
    masked     = attn + -1e8 * mask0[i]*mask1[j]
    a_m1 = softmax(masked over j);  a_m2 = softmax(masked over i)
    out0[i] = sum_j a_m1[i,j] q1[j];  out1[j] = sum_i a_m2[i,j] q0[i]

Sharding: data-parallel over batch B=16 -> 2 batches per core; params replicated.

Device strategy per (batch, oc) chunk of 128 output-features:
  PE:  s0T/s1T = W1-chunk.T @ q0T/q1T, then scores += W2oc.T @ tanh(...)
       as [128,1]x[128,512] matvecs accumulated over the 8 oc chunks in PSUM;
       the additive -1e8 mask is folded into the chains as [1,1]x[1,512]
       accumulate matmuls (spread one per chunk so the PE queue never
       convoys), so the epilogue needs no separate mask pass.
  DVE: outer-sum tmp[o',(i,j)] = (s1T[o',i]+b1[o']) + s0T[o',j] in fp16 with
       both operands laid out so every 32-bit read is a packed fp16 pair
       (s1T duplicated as [v|v] via a per-partition-bias tensor_scalar that
       also applies b1), making the tensor_tensor eligible for 2x_1P mode.
  ACT: h = tanh(tmp) - the dominant cost (~8.4M transcendentals per core);
       ACT does nothing else in steady state.
Epilogue per batch: bf16 exp straight off the PSUM score banks (mask already
applied), row/col sums via DVE reduce + PE transpose, normalization folded
into the final bmms as per-partition scales; batch 0's epilogue is emitted
in four phases interleaved with batch 1's chunks so no engine queue blocks.
"""

import numpy as np

B, L, D, O = 16, 64, 512, 1024
N_CORES = 8
BPC = B // N_CORES  # batches per core
OC = O // 128  # 8 o-chunks
DC = D // 128  # 4 d-chunks
NT = (L * L) // 512  # 8 ij-tiles of 512 per batch

_CACHE = {}


def _build_nc(debug=False):
    import concourse.mybir as mybir
    import concourse.tile as tile
    from concourse import bacc
    from concourse.masks import make_identity

    f32 = mybir.dt.float32
    f32r = mybir.dt.float32r
    f16 = mybir.dt.float16
    bf16 = mybir.dt.bfloat16
    AF = mybir.ActivationFunctionType

    nc = bacc.Bacc("TRN2", target_bir_lowering=False)

    q0n = nc.dram_tensor("q0n", [L, BPC, D], bf16, kind="ExternalInput")
    q1n = nc.dram_tensor("q1n", [L, BPC, D], bf16, kind="ExternalInput")
    q0t = nc.dram_tensor("q0t", [128, BPC, DC, L], bf16, kind="ExternalInput")
    q1t = nc.dram_tensor("q1t", [128, BPC, DC, L], bf16, kind="ExternalInput")
    w1t = nc.dram_tensor("w1t", [OC, 128, 2 * DC, 128], bf16, kind="ExternalInput")
    b1t = nc.dram_tensor("b1t", [128, OC], f32, kind="ExternalInput")
    w2t = nc.dram_tensor("w2t", [128, OC], bf16, kind="ExternalInput")
    wmf = nc.dram_tensor("wmf", [1, BPC * L * L], bf16, kind="ExternalInput")
    out0 = nc.dram_tensor("out0", [BPC, L, D], bf16, kind="ExternalOutput")
    out1 = nc.dram_tensor("out1", [BPC, L, D], bf16, kind="ExternalOutput")

    with tile.TileContext(nc) as tc:
        with (
            tc.tile_pool(name="consts", bufs=1) as consts,
            tc.tile_pool(name="qdata", bufs=1) as qdata,
            tc.tile_pool(name="st", bufs=3) as st_pool,
            tc.tile_pool(name="tmp", bufs=3) as tmp_pool,
            tc.tile_pool(name="hbuf", bufs=3) as h_pool,
            tc.tile_pool(name="ep", bufs=2) as ep_pool,
            tc.tile_pool(name="outb", bufs=2) as out_pool,
            tc.tile_pool(name="ps_scores", bufs=2, space="PSUM") as ps_scores,
            tc.tile_pool(name="ps_st", bufs=1, space="PSUM") as ps_st,
        ):
            # ---- inputs, in dependency order, all on the sync queue so the
            # shared DMA engines serve the critical path first; every source
            # is pre-tiled on host so each DMA is a contiguous HBM read
            # chunk 0's working set fans out over three queues in parallel
            q0ts = qdata.tile([128, BPC, DC, L], bf16, tag="q0ts")
            nc.scalar.dma_start(out=q0ts[:, 0:1], in_=q0t[:, 0:1])
            q1ts = qdata.tile([128, BPC, DC, L], bf16, tag="q1ts")
            nc.gpsimd.dma_start(out=q1ts[:, 0:1], in_=q1t[:, 0:1])
            w1s = [
                consts.tile([128, 2 * DC, 128], bf16, tag=f"w1s{oc}", name=f"w1s{oc}")
                for oc in range(OC)
            ]
            nc.sync.dma_start(out=w1s[0][:], in_=w1t[0])
            b1s = consts.tile([128, OC], f32)
            nc.scalar.dma_start(out=b1s[:], in_=b1t[:])
            w2s = consts.tile([128, OC], bf16)
            nc.scalar.dma_start(out=w2s[:], in_=w2t[:])
            nc.sync.dma_start(out=q0ts[:, 1:2], in_=q0t[:, 1:2])
            nc.sync.dma_start(out=q1ts[:, 1:2], in_=q1t[:, 1:2])
            for oc in range(1, 3):
                nc.sync.dma_start(out=w1s[oc][:], in_=w1t[oc])
            wmfs = consts.tile([1, BPC * L * L], bf16)
            nc.sync.dma_start(out=wmfs[:], in_=wmf[:])
            ones_bf = consts.tile([1, 1], bf16)
            nc.gpsimd.memset(ones_bf[:], 1.0)
            nc.sync.dma_start(out=w1s[3][:], in_=w1t[3])
            # epilogue-only tensors: issued mid-stream so they don't compete
            # with the first chunks' weights, but land well before batch 0's
            # epilogue (~10 chunks in)
            q0ns_all = qdata.tile([L, BPC, D], bf16, tag="q0ns")
            nc.sync.dma_start(out=q0ns_all[:], in_=q0n[:])
            q1ns_all = qdata.tile([L, BPC, D], bf16, tag="q1ns")
            nc.sync.dma_start(out=q1ns_all[:], in_=q1n[:])
            for oc in range(4, OC):
                nc.sync.dma_start(out=w1s[oc][:], in_=w1t[oc])
            q0ns = [q0ns_all[:, b, :] for b in range(BPC)]
            q1ns = [q1ns_all[:, b, :] for b in range(BPC)]

            ident = consts.tile([128, 128], f32)
            make_identity(nc, ident[:])
            identb = consts.tile([L, L], bf16)
            nc.vector.tensor_copy(identb[:], ident[0:L, 0:L])

            def w1sl(dc, oc):
                return w1s[oc][:, dc, :]

            # scores accumulators: 8 matvec chains per batch in ONE 3-bank
            # tile [65, 1536]. Chain t lives at partition row 32*(t%3) (bf16
            # matmuls support PSUM column groups {0,32,64}), column bank
            # t//3. After the exps consume a batch's banks, the same banks
            # are reused for that batch's epilogue matmuls.
            scores = [
                ps_scores.tile([65, 1536], f32, tag="scores", name=f"scores{b}")
                for b in range(BPC)
            ]

            def score_chain(b, t):
                g, a = t % 3, t // 3
                return scores[b][32 * g : 32 * g + 1, 512 * a : 512 * (a + 1)]

            # ---- main pipeline ----
            # Flat chunk loop with cross-batch software pipelining: the PE
            # queue must never block on the current chunk's tanh, so each
            # chunk's matvec is emitted one chunk later, and batch b's
            # epilogue is emitted in phases starting after batch b+1's
            # first chunk.
            def emit_mask_mm(pb, t):
                nc.tensor.matmul(
                    out=score_chain(pb, t),
                    lhsT=ones_bf[:],
                    rhs=wmfs[0:1, pb * L * L + t * 512 : pb * L * L + (t + 1) * 512],
                    start=False,
                    stop=False,
                )

            def emit_matvec(pb, poc, hs):
                per = NT // len(hs)
                for t in range(NT):
                    ph, off = hs[t // per], (t % per) * 512
                    nc.tensor.matmul(
                        out=score_chain(pb, t),
                        lhsT=w2s[:, poc : poc + 1],
                        rhs=ph[:, off : off + 512],
                        start=(poc == 0),
                        stop=(poc == OC - 1),
                    )
                # mask contributions: one tile per chunk, spread over the
                # chain so the PE queue never sees a burst
                if poc >= 1:
                    for t in range(poc - 1, NT, OC - 1):
                        emit_mask_mm(pb, t)

            def emit_outer_add(stj, std, tmp3, i_lo, i_hi):
                # tmp[o', (i, j)] = s0T[o', j] + (s1T[o', i] + b1[o'])
                # packed-pair fp16 views: every 32-bit read is [v_lo|v_hi]
                # with innermost step +1 on both sources -> DVE 2x_1P mode
                ni = i_hi - i_lo
                dst = tmp3[:, i_lo * L : i_hi * L].rearrange(
                    "p (i jp e) -> p i jp e", i=ni, jp=L // 2
                )
                in_j = (
                    stj[:]
                    .rearrange("p (jp e) -> p jp e", jp=L // 2)
                    .unsqueeze(1)
                    .broadcast_to((128, ni, L // 2, 2))
                )
                in_i = (
                    std[:, i_lo * 2 : i_hi * 2]
                    .rearrange("p (i e) -> p i e", i=ni)
                    .unsqueeze(2)
                    .broadcast_to((128, ni, L // 2, 2))
                )
                nc.vector.tensor_add(dst, in_j, in_i)

            attns = {}
            stgs = {}
            eps = {}

            def _stg(b):
                if b not in stgs:
                    stgs[b] = ep_pool.tile(
                        [65, 1536], bf16, tag="stg", name=f"stg{b}"
                    )
                    attns[b] = ep_pool.tile([L, L], bf16, tag="attn", name=f"attn{b}")
                return stgs[b]

            def emit_gather(b, a, eng):
                nchain = 3 if a < 2 else 2
                eng.dma_start(
                    out=attns[b][a * 24 : a * 24 + 8 * nchain, :],
                    in_=_stg(b)[0 : 32 * (nchain - 1) + 1 : 32, 512 * a : 512 * (a + 1)],
                )

            def emit_exp_gather(b, a, eng):
                # bf16 exp straight off one PSUM score bank (mask already
                # folded in; garbage rows between the chains are exp'd but
                # never read), then gather its chains with one strided DMA.
                sg = _stg(b)
                nc.scalar.activation(
                    sg[:, 512 * a : 512 * (a + 1)],
                    scores[b][:, 512 * a : 512 * (a + 1)],
                    AF.Exp,
                )
                emit_gather(b, a, eng)

            def emit_epilogue_a(b):
                # mid-stream batch: one exp instruction over all three banks
                nc.scalar.activation(_stg(b)[:], scores[b][:], AF.Exp)
                for a, eng in ((0, nc.sync), (1, nc.gpsimd), (2, nc.sync)):
                    emit_gather(b, a, eng)

            def emit_epilogue_b1(b):
                # row sums + transpose; the epilogue's PSUM lives in the
                # batch's own (already exp-consumed) score banks: bank 2
                # bitcast-bf16 holds the transpose, banks 0/1 the out bmms
                em = attns[b]
                sc_bf = scores[b].bitcast(bf16)
                emt_ps = sc_bf[0:L, 2048 : 2048 + L]
                nc.tensor.transpose(emt_ps, em[:], identb[:])
                rs = ep_pool.tile([L, 1], f32, tag="rs", name=f"rs{b}")
                nc.vector.reduce_sum(rs[:], em[:], axis=mybir.AxisListType.X)
                rrecip = ep_pool.tile([L, 1], f32, tag="rrecip", name=f"rrecip{b}")
                nc.vector.reciprocal(rrecip[:], rs[:])
                emt = ep_pool.tile([L, L], bf16, tag="emt", name=f"emt{b}")
                nc.vector.tensor_copy(emt[:], emt_ps)
                cs = ep_pool.tile([L, 1], f32, tag="cs", name=f"cs{b}")
                nc.vector.reduce_sum(cs[:], emt[:], axis=mybir.AxisListType.X)
                crecip = ep_pool.tile([L, 1], f32, tag="crecip", name=f"crecip{b}")
                nc.vector.reciprocal(crecip[:], cs[:])
                eps[b] = (em, emt, rrecip, crecip)

            def emit_epilogue_b2(b):
                # out1 = em^T-weighted q0, out0 = emt-weighted q1: both bmms
                # first, writing into the batch's consumed score banks 1, 0
                em, emt, rrecip, crecip = eps[b]
                o1_ps = ps_st.tile([L, D], f32, tag="o1ps", name=f"o1_ps{b}")[:]
                nc.tensor.matmul(
                    out=o1_ps, lhsT=em[:], rhs=q0ns[b], start=True, stop=True
                )
                o0_ps = scores[b][0:L, 0:512]
                nc.tensor.matmul(
                    out=o0_ps, lhsT=emt[:], rhs=q1ns[b], start=True, stop=True
                )
                eps[b] = (o0_ps, o1_ps, rrecip, crecip)

            def emit_epilogue_b3(b):
                # out = recip-scaled bmm results; the two scale multiplies
                # run on different engines (ACT is idle after the last exp)
                o0_ps, o1_ps, rrecip, crecip = eps[b]
                o0_sb = out_pool.tile([L, D], bf16, tag="o0_sb", name=f"o0_sb{b}")
                nc.vector.tensor_scalar_mul(o0_sb[:], o0_ps, rrecip[:])
                nc.sync.dma_start(out=out0[b], in_=o0_sb[:])
                o1_sb = out_pool.tile([L, D], bf16, tag="o1_sb", name=f"o1_sb{b}")
                if b == BPC - 1:
                    nc.scalar.activation(o1_sb[:], o1_ps, AF.Copy, scale=crecip[:])
                    nc.scalar.dma_start(out=out1[b], in_=o1_sb[:])
                else:
                    nc.vector.tensor_scalar_mul(o1_sb[:], o1_ps, crecip[:])
                    nc.sync.dma_start(out=out1[b], in_=o1_sb[:])

            pending = None
            chunks = [(b, oc) for b in range(BPC) for oc in range(OC)]
            for ci, (b, oc) in enumerate(chunks):
                # s0T / s1T for this o-chunk: PSUM [128, 128]
                st_ps = ps_st.tile([128, 128], f32, tag="st_ps")
                for dc in range(DC):
                    nc.tensor.matmul(
                        out=st_ps[:, 0:64],
                        lhsT=w1sl(dc, oc),
                        rhs=q0ts[:, b, dc, :],
                        start=(dc == 0),
                        stop=(dc == DC - 1),
                    )
                # stj only needs the s0 half: emit its copy before the s1
                # matmuls so chunk 0's critical chain starts 4 matmuls sooner
                stj = st_pool.tile([128, L], f16, tag="stj")
                nc.vector.tensor_copy(stj[:], st_ps[:, 0:64])
                for dc in range(DC):
                    nc.tensor.matmul(
                        out=st_ps[:, 64:128],
                        lhsT=w1sl(DC + dc, oc),
                        rhs=q1ts[:, b, dc, :],
                        start=(dc == 0),
                        stop=(dc == DC - 1),
                    )
                # duplicate s1T as [v|v] pairs for packed reads, folding the
                # b1 bias in as a per-partition scalar so tanh needs no bias
                std = st_pool.tile([128, 2 * L], f16, tag="std")
                nc.vector.tensor_scalar_add(
                    std[:].rearrange("p (i e) -> p i e", i=L),
                    st_ps[:, 64:128].unsqueeze(2).broadcast_to((128, L, 2)),
                    b1s[:, oc : oc + 1],
                )

                if ci == 0:
                    # first chunk: quarter-split so the ACT stream starts as
                    # soon as the first quarter's outer-add lands
                    hq = []
                    tmp3 = tmp_pool.tile([128, L * L], f16, tag="tmp3")
                    for q in range(4):
                        emit_outer_add(stj, std, tmp3, q * (L // 4), (q + 1) * (L // 4))
                        qf = slice(q * (L * L // 4), (q + 1) * (L * L // 4))
                        hqt = h_pool.tile(
                            [128, L * L // 4], bf16, tag=f"hq{q}", name=f"hq{q}"
                        )
                        nc.scalar.activation(hqt[:], tmp3[:, qf], AF.Tanh)
                        hq.append(hqt)
                    pending = (b, oc, tuple(hq))
                    continue

                if ci == len(chunks) - 1:
                    # final chunk: reverse-order quarters with per-quarter
                    # matvecs, exps and gathers interleaved into the ACT
                    # stream so score bank 2 (tiles 6,7) drains first and
                    # the epilogue's serial chain starts ~4us earlier
                    emit_matvec(*pending)
                    pending = None
                    tmp3 = tmp_pool.tile([128, L * L], f16, tag="tmp3")
                    qtiles = {3: (6, 7), 2: (4, 5), 1: (3, 2), 0: (1, 0)}
                    for q in (3, 2, 1, 0):
                        emit_outer_add(stj, std, tmp3, q * (L // 4), (q + 1) * (L // 4))
                        qf = slice(q * (L * L // 4), (q + 1) * (L * L // 4))
                        hqt = h_pool.tile(
                            [128, L * L // 4], bf16, tag=f"hq{q}", name=f"hq{q}"
                        )
                        nc.scalar.activation(hqt[:], tmp3[:, qf], AF.Tanh)
                        for t in qtiles[q]:
                            nc.tensor.matmul(
                                out=score_chain(b, t),
                                lhsT=w2s[:, oc : oc + 1],
                                rhs=hqt[:, (t - 2 * q) * 512 : (t - 2 * q + 1) * 512],
                                start=False,
                                stop=True,
                            )
                            if t == 6:
                                emit_mask_mm(b, 6)
                        if q == 1:
                            emit_exp_gather(b, 2, nc.sync)
                    emit_exp_gather(b, 1, nc.gpsimd)
                    emit_exp_gather(b, 0, nc.sync)
                    continue

                tmp3 = tmp_pool.tile([128, L * L], f16, tag="tmp3")
                emit_outer_add(stj, std, tmp3, 0, L)
                h3 = h_pool.tile([128, L * L], bf16, tag="h3")
                nc.scalar.activation(h3[:], tmp3[:], AF.Tanh)

                # matvec for the PREVIOUS chunk (software pipelining)
                if pending is not None:
                    emit_matvec(*pending)
                pending = (b, oc, (h3,))

                # batch b-1's epilogue lands a few chunks into batch b, in
                # phases so no engine queue blocks on a cross-engine chain
                if ci == OC + 1:
                    emit_epilogue_a(0)
                if ci == OC + 3:
                    emit_epilogue_b1(0)
                if ci == OC + 5:
                    emit_epilogue_b2(0)
                if ci == OC + 6:
                    emit_epilogue_b3(0)

            emit_epilogue_b1(BPC - 1)
            emit_epilogue_b2(BPC - 1)
            emit_epilogue_b3(BPC - 1)

    nc.finalize()
    return nc


def _get_nc():
    if "nc" not in _CACHE:
        _CACHE["nc"] = _build_nc()
    return _CACHE["nc"]


def build_in_maps(q0, q1, mask0, mask1, W1, b1, W2, b2):
    import ml_dtypes

    q0 = np.asarray(q0, dtype=np.float32)
    q1 = np.asarray(q1, dtype=np.float32)
    W1 = np.ascontiguousarray(np.asarray(W1, dtype=np.float32))
    b1 = np.asarray(b1, dtype=np.float32)
    W2 = np.asarray(W2, dtype=np.float32)
    m0f = np.asarray(mask0).astype(np.float32)
    m1f = np.asarray(mask1).astype(np.float32)

    b1t = np.ascontiguousarray(b1.reshape(OC, 128).T)
    w2t = np.ascontiguousarray(W2[:, 0].reshape(OC, 128).T).astype(ml_dtypes.bfloat16)
    # w1t[oc, p, c, o] = W1[c*128+p, oc*128+o]
    w1t = np.ascontiguousarray(
        W1.astype(ml_dtypes.bfloat16).reshape(8, 128, OC, 128).transpose(2, 1, 0, 3)
    )
    # additive mask in scores-flat order: -1e8 * m0[i] * m1[j]
    wm_add = (-1e8) * (m0f[:, :, None] * m1f[:, None, :])  # [B, L, L]

    in_maps = []
    for c in range(N_CORES):
        sl = slice(BPC * c, BPC * (c + 1))
        q0c = np.ascontiguousarray(q0[sl])
        q1c = np.ascontiguousarray(q1[sl])

        def qtile(qc):
            # [p, b, dc, l] = qc[b, l, dc*128+p]
            return np.ascontiguousarray(
                qc.transpose(2, 0, 1).reshape(DC, 128, BPC, L).transpose(1, 2, 0, 3)
            ).astype(ml_dtypes.bfloat16)

        in_maps.append(
            {
                "q0n": np.ascontiguousarray(q0c.transpose(1, 0, 2)).astype(
                    ml_dtypes.bfloat16
                ),
                "q1n": np.ascontiguousarray(q1c.transpose(1, 0, 2)).astype(
                    ml_dtypes.bfloat16
                ),
                "q0t": qtile(q0c),
                "q1t": qtile(q1c),
                "w1t": w1t,
                "b1t": b1t,
                "w2t": w2t,
                "wmf": np.ascontiguousarray(wm_add[sl].reshape(1, BPC * L * L)).astype(
                    ml_dtypes.bfloat16
                ),
            }
        )
    return in_maps


def kernel(q0, q1, mask0, mask1, W1, b1, W2, b2, **_unused):
    from concourse.bass_utils import run_bass_kernel_spmd

    in_maps = build_in_maps(q0, q1, mask0, mask1, W1, b1, W2, b2)
    nc = _get_nc()
    res = run_bass_kernel_spmd(nc, in_maps, core_ids=list(range(N_CORES)))
    out0 = np.concatenate(
        [np.asarray(res.results[c]["out0"], np.float32) for c in range(N_CORES)], axis=0
    )
    out1 = np.concatenate(
        [np.asarray(res.results[c]["out1"], np.float32) for c in range(N_CORES)], axis=0
    )
    return out0, out1


# revision 26
# speedup vs baseline: 1.0370x; 1.0370x over previous
"""Bahdanau additive attention between two sequences on 8 Trainium2 NeuronCores.

Reference computation (per batch b):
    s0 = q0 @ W1[:D]          # [L, O]
    s1 = q1 @ W1[D:]          # [L, O]
    h[i,j,:]   = tanh(s1[i] + s0[j] + b1)          # [L, L, O]
    attn[i,j]  = h[i,j,:] . W2 (+ b2, drops out of softmax)
    masked     = attn + -1e8 * mask0[i]*mask1[j]
    a_m1 = softmax(masked over j);  a_m2 = softmax(masked over i)
    out0[i] = sum_j a_m1[i,j] q1[j];  out1[j] = sum_i a_m2[i,j] q0[i]

Sharding: data-parallel over batch B=16 -> 2 batches per core; params replicated.

Device strategy per (batch, oc) chunk of 128 output-features:
  PE:  s0T/s1T = W1-chunk.T @ q0T/q1T, then scores += W2oc.T @ tanh(...)
       as [128,1]x[128,512] matvecs accumulated over the 8 oc chunks in PSUM;
       the additive -1e8 mask is folded into the chains as [1,1]x[1,512]
       accumulate matmuls (spread one per chunk so the PE queue never
       convoys), so the epilogue needs no separate mask pass.
  DVE: outer-sum tmp[o',(i,j)] = (s1T[o',i]+b1[o']) + s0T[o',j] in fp16 with
       both operands laid out so every 32-bit read is a packed fp16 pair
       (s1T duplicated as [v|v] via a per-partition-bias tensor_scalar that
       also applies b1), making the tensor_tensor eligible for 2x_1P mode.
  ACT: h = tanh(tmp) - the dominant cost (~8.4M transcendentals per core);
       ACT does nothing else in steady state.
Epilogue per batch: bf16 exp straight off the PSUM score banks (mask already
applied), row/col sums via DVE reduce + PE transpose, normalization folded
into the final bmms as per-partition scales; batch 0's epilogue is emitted
in four phases interleaved with batch 1's chunks so no engine queue blocks.
"""

import numpy as np

B, L, D, O = 16, 64, 512, 1024
N_CORES = 8
BPC = B // N_CORES  # batches per core
OC = O // 128  # 8 o-chunks
DC = D // 128  # 4 d-chunks
NT = (L * L) // 512  # 8 ij-tiles of 512 per batch

_CACHE = {}


def _build_nc(debug=False):
    import concourse.mybir as mybir
    import concourse.tile as tile
    from concourse import bacc
    from concourse.masks import make_identity

    f32 = mybir.dt.float32
    f32r = mybir.dt.float32r
    f16 = mybir.dt.float16
    bf16 = mybir.dt.bfloat16
    AF = mybir.ActivationFunctionType

    nc = bacc.Bacc("TRN2", target_bir_lowering=False)

    q0n = nc.dram_tensor("q0n", [L, BPC, D], bf16, kind="ExternalInput")
    q1n = nc.dram_tensor("q1n", [L, BPC, D], bf16, kind="ExternalInput")
    q0t = nc.dram_tensor("q0t", [128, BPC, DC, L], bf16, kind="ExternalInput")
    q1t = nc.dram_tensor("q1t", [128, BPC, DC, L], bf16, kind="ExternalInput")
    w1t = nc.dram_tensor("w1t", [OC, 128, 2 * DC, 128], bf16, kind="ExternalInput")
    b1t = nc.dram_tensor("b1t", [128, OC], f32, kind="ExternalInput")
    w2t = nc.dram_tensor("w2t", [128, OC], bf16, kind="ExternalInput")
    wmf = nc.dram_tensor("wmf", [1, BPC * L * L], bf16, kind="ExternalInput")
    out0 = nc.dram_tensor("out0", [BPC, L, D], bf16, kind="ExternalOutput")
    out1 = nc.dram_tensor("out1", [BPC, L, D], bf16, kind="ExternalOutput")

    with tile.TileContext(nc) as tc:
        with (
            tc.tile_pool(name="consts", bufs=1) as consts,
            tc.tile_pool(name="qdata", bufs=1) as qdata,
            tc.tile_pool(name="st", bufs=3) as st_pool,
            tc.tile_pool(name="tmp", bufs=3) as tmp_pool,
            tc.tile_pool(name="hbuf", bufs=3) as h_pool,
            tc.tile_pool(name="ep", bufs=2) as ep_pool,
            tc.tile_pool(name="outb", bufs=2) as out_pool,
            tc.tile_pool(name="ps_scores", bufs=2, space="PSUM") as ps_scores,
            tc.tile_pool(name="ps_st", bufs=1, space="PSUM") as ps_st,
        ):
            # ---- inputs, in dependency order, all on the sync queue so the
            # shared DMA engines serve the critical path first; every source
            # is pre-tiled on host so each DMA is a contiguous HBM read
            # chunk 0's working set fans out over three queues in parallel
            q0ts = qdata.tile([128, BPC, DC, L], bf16, tag="q0ts")
            nc.scalar.dma_start(out=q0ts[:, 0:1], in_=q0t[:, 0:1])
            q1ts = qdata.tile([128, BPC, DC, L], bf16, tag="q1ts")
            nc.gpsimd.dma_start(out=q1ts[:, 0:1], in_=q1t[:, 0:1])
            w1s = [
                consts.tile([128, 2 * DC, 128], bf16, tag=f"w1s{oc}", name=f"w1s{oc}")
                for oc in range(OC)
            ]
            nc.sync.dma_start(out=w1s[0][:], in_=w1t[0])
            b1s = consts.tile([128, OC], f32)
            nc.scalar.dma_start(out=b1s[:], in_=b1t[:])
            w2s = consts.tile([128, OC], bf16)
            nc.scalar.dma_start(out=w2s[:], in_=w2t[:])
            nc.sync.dma_start(out=q0ts[:, 1:2], in_=q0t[:, 1:2])
            nc.sync.dma_start(out=q1ts[:, 1:2], in_=q1t[:, 1:2])
            for oc in range(1, 3):
                nc.sync.dma_start(out=w1s[oc][:], in_=w1t[oc])
            wmfs = consts.tile([1, BPC * L * L], bf16)
            nc.sync.dma_start(out=wmfs[:], in_=wmf[:])
            ones_bf = consts.tile([1, 1], bf16)
            nc.gpsimd.memset(ones_bf[:], 1.0)
            nc.sync.dma_start(out=w1s[3][:], in_=w1t[3])
            # epilogue-only tensors: issued mid-stream so they don't compete
            # with the first chunks' weights, but land well before batch 0's
            # epilogue (~10 chunks in)
            q0ns_all = qdata.tile([L, BPC, D], bf16, tag="q0ns")
            nc.sync.dma_start(out=q0ns_all[:], in_=q0n[:])
            q1ns_all = qdata.tile([L, BPC, D], bf16, tag="q1ns")
            nc.sync.dma_start(out=q1ns_all[:], in_=q1n[:])
            for oc in range(4, OC):
                nc.sync.dma_start(out=w1s[oc][:], in_=w1t[oc])
            q0ns = [q0ns_all[:, b, :] for b in range(BPC)]
            q1ns = [q1ns_all[:, b, :] for b in range(BPC)]

            ident = consts.tile([128, 128], f32)
            make_identity(nc, ident[:])
            identb = consts.tile([L, L], bf16)
            nc.vector.tensor_copy(identb[:], ident[0:L, 0:L])

            def w1sl(dc, oc):
                return w1s[oc][:, dc, :]

            # scores accumulators: 8 matvec chains per batch in ONE 3-bank
            # tile [65, 1536]. Chain t lives at partition row 32*(t%3) (bf16
            # matmuls support PSUM column groups {0,32,64}), column bank
            # t//3. After the exps consume a batch's banks, the same banks
            # are reused for that batch's epilogue matmuls.
            scores = [
                ps_scores.tile([65, 1536], f32, tag="scores", name=f"scores{b}")
                for b in range(BPC)
            ]

            def score_chain(b, t):
                g, a = t % 3, t // 3
                return scores[b][32 * g : 32 * g + 1, 512 * a : 512 * (a + 1)]

            # ---- main pipeline ----
            # Flat chunk loop with cross-batch software pipelining: the PE
            # queue must never block on the current chunk's tanh, so each
            # chunk's matvec is emitted one chunk later, and batch b's
            # epilogue is emitted in phases starting after batch b+1's
            # first chunk.
            def emit_mask_mm(pb, t):
                nc.tensor.matmul(
                    out=score_chain(pb, t),
                    lhsT=ones_bf[:],
                    rhs=wmfs[0:1, pb * L * L + t * 512 : pb * L * L + (t + 1) * 512],
                    start=False,
                    stop=False,
                )

            def emit_matvec(pb, poc, hs):
                per = NT // len(hs)
                for t in range(NT):
                    ph, off = hs[t // per], (t % per) * 512
                    nc.tensor.matmul(
                        out=score_chain(pb, t),
                        lhsT=w2s[:, poc : poc + 1],
                        rhs=ph[:, off : off + 512],
                        start=(poc == 0),
                        stop=(poc == OC - 1),
                    )
                # mask contributions: one tile per chunk, spread over the
                # chain so the PE queue never sees a burst
                if poc >= 1:
                    for t in range(poc - 1, NT, OC - 1):
                        emit_mask_mm(pb, t)

            def emit_outer_add(stj, std, tmp3, i_lo, i_hi):
                # tmp[o', (i, j)] = s0T[o', j] + (s1T[o', i] + b1[o'])
                # packed-pair fp16 views: every 32-bit read is [v_lo|v_hi]
                # with innermost step +1 on both sources -> DVE 2x_1P mode
                ni = i_hi - i_lo
                dst = tmp3[:, i_lo * L : i_hi * L].rearrange(
                    "p (i jp e) -> p i jp e", i=ni, jp=L // 2
                )
                in_j = (
                    stj[:]
                    .rearrange("p (jp e) -> p jp e", jp=L // 2)
                    .unsqueeze(1)
                    .broadcast_to((128, ni, L // 2, 2))
                )
                in_i = (
                    std[:, i_lo * 2 : i_hi * 2]
                    .rearrange("p (i e) -> p i e", i=ni)
                    .unsqueeze(2)
                    .broadcast_to((128, ni, L // 2, 2))
                )
                nc.vector.tensor_add(dst, in_j, in_i)

            attns = {}
            stgs = {}
            eps = {}

            def _stg(b):
                if b not in stgs:
                    stgs[b] = ep_pool.tile(
                        [65, 1536], bf16, tag="stg", name=f"stg{b}"
                    )
                    attns[b] = ep_pool.tile([L, L], bf16, tag="attn", name=f"attn{b}")
                return stgs[b]

            def emit_gather(b, a, eng):
                nchain = 3 if a < 2 else 2
                eng.dma_start(
                    out=attns[b][a * 24 : a * 24 + 8 * nchain, :],
                    in_=_stg(b)[0 : 32 * (nchain - 1) + 1 : 32, 512 * a : 512 * (a + 1)],
                )

            def emit_exp_gather(b, a, eng):
                # bf16 exp straight off one PSUM score bank (mask already
                # folded in; garbage rows between the chains are exp'd but
                # never read), then gather its chains with one strided DMA.
                sg = _stg(b)
                nc.scalar.activation(
                    sg[:, 512 * a : 512 * (a + 1)],
                    scores[b][:, 512 * a : 512 * (a + 1)],
                    AF.Exp,
                )
                emit_gather(b, a, eng)

            def emit_epilogue_a(b):
                # mid-stream batch: one exp instruction over all three banks
                nc.scalar.activation(_stg(b)[:], scores[b][:], AF.Exp)
                for a, eng in ((0, nc.sync), (1, nc.gpsimd), (2, nc.sync)):
                    emit_gather(b, a, eng)

            def emit_epilogue_b1(b):
                # row sums + transpose; the epilogue's PSUM lives in the
                # batch's own (already exp-consumed) score banks: bank 2
                # bitcast-bf16 holds the transpose, banks 0/1 the out bmms
                em = attns[b]
                sc_bf = scores[b].bitcast(bf16)
                emt_ps = sc_bf[0:L, 2048 : 2048 + L]
                nc.tensor.transpose(emt_ps, em[:], identb[:])
                rs = ep_pool.tile([L, 1], f32, tag="rs", name=f"rs{b}")
                nc.vector.reduce_sum(rs[:], em[:], axis=mybir.AxisListType.X)
                rrecip = ep_pool.tile([L, 1], f32, tag="rrecip", name=f"rrecip{b}")
                nc.vector.reciprocal(rrecip[:], rs[:])
                emt = ep_pool.tile([L, L], bf16, tag="emt", name=f"emt{b}")
                nc.vector.tensor_copy(emt[:], emt_ps)
                cs = ep_pool.tile([L, 1], f32, tag="cs", name=f"cs{b}")
                nc.vector.reduce_sum(cs[:], emt[:], axis=mybir.AxisListType.X)
                crecip = ep_pool.tile([L, 1], f32, tag="crecip", name=f"crecip{b}")
                nc.vector.reciprocal(crecip[:], cs[:])
                eps[b] = (em, emt, rrecip, crecip)

            def emit_epilogue_b2(b):
                # out1 = em^T-weighted q0, out0 = emt-weighted q1: both bmms
                # first, writing into the batch's consumed score banks 1, 0
                em, emt, rrecip, crecip = eps[b]
                o1_ps = ps_st.tile([L, D], f32, tag="o1ps", name=f"o1_ps{b}")[:]
                nc.tensor.matmul(
                    out=o1_ps, lhsT=em[:], rhs=q0ns[b], start=True, stop=True
                )
                o0_ps = scores[b][0:L, 0:512]
                nc.tensor.matmul(
                    out=o0_ps, lhsT=emt[:], rhs=q1ns[b], start=True, stop=True
                )
                eps[b] = (o0_ps, o1_ps, rrecip, crecip)

            def emit_epilogue_b3(b):
                # out = recip-scaled bmm results; the two scale multiplies
                # run on different engines (ACT is idle after the last exp)
                o0_ps, o1_ps, rrecip, crecip = eps[b]
                o0_sb = out_pool.tile([L, D], bf16, tag="o0_sb", name=f"o0_sb{b}")
                nc.vector.tensor_scalar_mul(o0_sb[:], o0_ps, rrecip[:])
                nc.sync.dma_start(out=out0[b], in_=o0_sb[:])
                o1_sb = out_pool.tile([L, D], bf16, tag="o1_sb", name=f"o1_sb{b}")
                if b == BPC - 1:
                    nc.scalar.activation(o1_sb[:], o1_ps, AF.Copy, scale=crecip[:])
                    nc.scalar.dma_start(out=out1[b], in_=o1_sb[:])
                else:
                    nc.vector.tensor_scalar_mul(o1_sb[:], o1_ps, crecip[:])
                    nc.sync.dma_start(out=out1[b], in_=o1_sb[:])

            pending = None
            chunks = [(b, oc) for b in range(BPC) for oc in range(OC)]
            for ci, (b, oc) in enumerate(chunks):
                # s0T / s1T for this o-chunk: PSUM [128, 128]
                st_ps = ps_st.tile([128, 128], f32, tag="st_ps")
                for dc in range(DC):
                    nc.tensor.matmul(
                        out=st_ps[:, 0:64],
                        lhsT=w1sl(dc, oc),
                        rhs=q0ts[:, b, dc, :],
                        start=(dc == 0),
                        stop=(dc == DC - 1),
                    )
                for dc in range(DC):
                    nc.tensor.matmul(
                        out=st_ps[:, 64:128],
                        lhsT=w1sl(DC + dc, oc),
                        rhs=q1ts[:, b, dc, :],
                        start=(dc == 0),
                        stop=(dc == DC - 1),
                    )
                stj = st_pool.tile([128, L], f16, tag="stj")
                nc.vector.tensor_copy(stj[:], st_ps[:, 0:64])
                # duplicate s1T as [v|v] pairs for packed reads, folding the
                # b1 bias in as a per-partition scalar so tanh needs no bias
                std = st_pool.tile([128, 2 * L], f16, tag="std")
                nc.vector.tensor_scalar_add(
                    std[:].rearrange("p (i e) -> p i e", i=L),
                    st_ps[:, 64:128].unsqueeze(2).broadcast_to((128, L, 2)),
                    b1s[:, oc : oc + 1],
                )

                if ci == 0:
                    # first chunk: quarter-split so the ACT stream starts as
                    # soon as the first quarter's outer-add lands
                    hq = []
                    tmp3 = tmp_pool.tile([128, L * L], f16, tag="tmp3")
                    for q in range(4):
                        emit_outer_add(stj, std, tmp3, q * (L // 4), (q + 1) * (L // 4))
                        qf = slice(q * (L * L // 4), (q + 1) * (L * L // 4))
                        hqt = h_pool.tile(
                            [128, L * L // 4], bf16, tag=f"hq{q}", name=f"hq{q}"
                        )
                        nc.scalar.activation(hqt[:], tmp3[:, qf], AF.Tanh)
                        hq.append(hqt)
                    pending = (b, oc, tuple(hq))
                    continue

                if ci == len(chunks) - 1:
                    # final chunk: reverse-order quarters with per-quarter
                    # matvecs, exps and gathers interleaved into the ACT
                    # stream so score bank 2 (tiles 6,7) drains first and
                    # the epilogue's serial chain starts ~4us earlier
                    emit_matvec(*pending)
                    pending = None
                    tmp3 = tmp_pool.tile([128, L * L], f16, tag="tmp3")
                    qtiles = {3: (6, 7), 2: (4, 5), 1: (3, 2), 0: (1, 0)}
                    for q in (3, 2, 1, 0):
                        emit_outer_add(stj, std, tmp3, q * (L // 4), (q + 1) * (L // 4))
                        qf = slice(q * (L * L // 4), (q + 1) * (L * L // 4))
                        hqt = h_pool.tile(
                            [128, L * L // 4], bf16, tag=f"hq{q}", name=f"hq{q}"
                        )
                        nc.scalar.activation(hqt[:], tmp3[:, qf], AF.Tanh)
                        for t in qtiles[q]:
                            nc.tensor.matmul(
                                out=score_chain(b, t),
                                lhsT=w2s[:, oc : oc + 1],
                                rhs=hqt[:, (t - 2 * q) * 512 : (t - 2 * q + 1) * 512],
                                start=False,
                                stop=True,
                            )
                            if t == 6:
                                emit_mask_mm(b, 6)
                        if q == 1:
                            emit_exp_gather(b, 2, nc.sync)
                    emit_exp_gather(b, 1, nc.gpsimd)
                    emit_exp_gather(b, 0, nc.sync)
                    continue

                tmp3 = tmp_pool.tile([128, L * L], f16, tag="tmp3")
                emit_outer_add(stj, std, tmp3, 0, L)
                h3 = h_pool.tile([128, L * L], bf16, tag="h3")
                nc.scalar.activation(h3[:], tmp3[:], AF.Tanh)

                # matvec for the PREVIOUS chunk (software pipelining)
                if pending is not None:
                    emit_matvec(*pending)
                pending = (b, oc, (h3,))

                # batch b-1's epilogue lands a few chunks into batch b, in
                # phases so no engine queue blocks on a cross-engine chain
                if ci == OC + 1:
                    emit_epilogue_a(0)
                if ci == OC + 3:
                    emit_epilogue_b1(0)
                if ci == OC + 5:
                    emit_epilogue_b2(0)
                if ci == OC + 6:
                    emit_epilogue_b3(0)

            emit_epilogue_b1(BPC - 1)
            emit_epilogue_b2(BPC - 1)
            emit_epilogue_b3(BPC - 1)

    nc.finalize()
    return nc


def _get_nc():
    if "nc" not in _CACHE:
        _CACHE["nc"] = _build_nc()
    return _CACHE["nc"]


def build_in_maps(q0, q1, mask0, mask1, W1, b1, W2, b2):
    import ml_dtypes

    q0 = np.asarray(q0, dtype=np.float32)
    q1 = np.asarray(q1, dtype=np.float32)
    W1 = np.ascontiguousarray(np.asarray(W1, dtype=np.float32))
    b1 = np.asarray(b1, dtype=np.float32)
    W2 = np.asarray(W2, dtype=np.float32)
    m0f = np.asarray(mask0).astype(np.float32)
    m1f = np.asarray(mask1).astype(np.float32)

    b1t = np.ascontiguousarray(b1.reshape(OC, 128).T)
    w2t = np.ascontiguousarray(W2[:, 0].reshape(OC, 128).T).astype(ml_dtypes.bfloat16)
    # w1t[oc, p, c, o] = W1[c*128+p, oc*128+o]
    w1t = np.ascontiguousarray(
        W1.astype(ml_dtypes.bfloat16).reshape(8, 128, OC, 128).transpose(2, 1, 0, 3)
    )
    # additive mask in scores-flat order: -1e8 * m0[i] * m1[j]
    wm_add = (-1e8) * (m0f[:, :, None] * m1f[:, None, :])  # [B, L, L]

    in_maps = []
    for c in range(N_CORES):
        sl = slice(BPC * c, BPC * (c + 1))
        q0c = np.ascontiguousarray(q0[sl])
        q1c = np.ascontiguousarray(q1[sl])

        def qtile(qc):
            # [p, b, dc, l] = qc[b, l, dc*128+p]
            return np.ascontiguousarray(
                qc.transpose(2, 0, 1).reshape(DC, 128, BPC, L).transpose(1, 2, 0, 3)
            ).astype(ml_dtypes.bfloat16)

        in_maps.append(
            {
                "q0n": np.ascontiguousarray(q0c.transpose(1, 0, 2)).astype(
                    ml_dtypes.bfloat16
                ),
                "q1n": np.ascontiguousarray(q1c.transpose(1, 0, 2)).astype(
                    ml_dtypes.bfloat16
                ),
                "q0t": qtile(q0c),
                "q1t": qtile(q1c),
                "w1t": w1t,
                "b1t": b1t,
                "w2t": w2t,
                "wmf": np.ascontiguousarray(wm_add[sl].reshape(1, BPC * L * L)).astype(
                    ml_dtypes.bfloat16
                ),
            }
        )
    return in_maps


def kernel(q0, q1, mask0, mask1, W1, b1, W2, b2, **_unused):
    from concourse.bass_utils import run_bass_kernel_spmd

    in_maps = build_in_maps(q0, q1, mask0, mask1, W1, b1, W2, b2)
    nc = _get_nc()
    res = run_bass_kernel_spmd(nc, in_maps, core_ids=list(range(N_CORES)))
    out0 = np.concatenate(
        [np.asarray(res.results[c]["out0"], np.float32) for c in range(N_CORES)], axis=0
    )
    out1 = np.concatenate(
        [np.asarray(res.results[c]["out1"], np.float32) for c in range(N_CORES)], axis=0
    )
    return out0, out1
